# revision 1
# baseline (speedup 1.0000x reference)
"""CNN-LSTM Trainium2 kernel (nn_CNN_LSTM_41205916238256).

Pipeline per core (batch-parallel, 32 batch elems per core):
  1. Embedding gather via indirect DMA (emb table pre-cast to bf16).
  2. PE transposes -> embT [E, tokens] bf16.
  3. Conv(width 5 over seq, full E contraction) as 10 accumulated matmuls
     per (batch, nf-half); ReLU+bias on ACT -> convT [NF, t-major] bf16.
  4. Input projection Xp = relu_conv @ W_ih_eff^T + bias_eff, stored
     [128, t*128 + g*32 + b] bf16.
  5. 508-step LSTM recurrence, transposed layout (H on partitions):
       bank_t = U_eff @ r_{t-1} (4 MMs, start) + Xp_t (identity inject)
       S = sigmoid(bank)                        [f,i,o,g at cols 0/32/64/96]
       t1 = (S_g - .5) * S_i ; t2 = S_f * P ; P' = 4*t1 + t2
       sigP = sigmoid(P') ; r = (sigP - .5) * S_o   (bf16)
     with P == 2c, r == h/2; prescales folded into weights on host:
       W_ih/bias rows: g x2;  w_hh rows: f,i,o x2, g x4.
  6. h_n = 2*(sigP-0.5)*S_o in fp32, DMA out transposed [128, 32].
"""
import numpy as np
import ml_dtypes

import concourse.bacc as bacc
import concourse.bass as bass
import concourse.mybir as mybir
import concourse.tile as tile
from concourse.bass_utils import run_bass_kernel_spmd

BF16 = mybir.dt.bfloat16
F32 = mybir.dt.float32
I32 = mybir.dt.int32
AF = mybir.ActivationFunctionType
OP = mybir.AluOpType

VOCAB, EMB, KER, NF, HID = 50257, 256, 5, 256, 128
B, S = 256, 512
T = S - KER + 1            # 508
NC = 8                     # cores
BL = B // NC               # 32 batch per core
P = 128
CB = 4                     # batch chunk in prologue
NCH = BL // CB             # 8 chunks
TQ = S // P                # 4 128-token groups per batch elem

_PROGRAM = None


def _build_program(debug=False):
    nc = bacc.Bacc("TRN2", target_bir_lowering=False, debug=False)

    emb_d = nc.dram_tensor("embt", [VOCAB, EMB], BF16, kind="ExternalInput")
    idx_d = nc.dram_tensor("idx", [P, BL * TQ], I32, kind="ExternalInput")
    cw_d = nc.dram_tensor("cw", [KER * 2 * 2, P, P], BF16, kind="ExternalInput")
    cb_d = nc.dram_tensor("cb", [P, 2], F32, kind="ExternalInput")
    wih_d = nc.dram_tensor("wih", [4 * 2, P, P], BF16, kind="ExternalInput")
    be_d = nc.dram_tensor("be", [P, 4], F32, kind="ExternalInput")
    u_d = nc.dram_tensor("u", [4, P, P], BF16, kind="ExternalInput")
    eye_d = nc.dram_tensor("eye", [P, P], BF16, kind="ExternalInput")
    r0_d = nc.dram_tensor("r0", [P, BL], BF16, kind="ExternalInput")
    hT_d = nc.dram_tensor("hT", [P, BL], F32, kind="ExternalOutput")
    if debug:
        embT_dump = nc.dram_tensor("embT_dump", [2, P, CB * S], BF16, kind="ExternalOutput")
        convT_dump = nc.dram_tensor("convT_dump", [2, P, CB * T], BF16, kind="ExternalOutput")
        xp_dump = nc.dram_tensor("xp_dump", [P, T * P], BF16, kind="ExternalOutput")
        s0_dump = nc.dram_tensor("s0_dump", [P, P], F32, kind="ExternalOutput")
        g_dump = nc.dram_tensor("g_dump", [P, CB * TQ * EMB], BF16, kind="ExternalOutput")
        r1_dump = nc.dram_tensor("r1_dump", [P, BL], F32, kind="ExternalOutput")

    with tile.TileContext(nc) as tc:
        with tc.tile_pool(name="stat", bufs=1) as stat:
            # ---- static loads
            idx_t = stat.tile([P, BL * TQ], I32, tag="idx")
            nc.sync.dma_start(out=idx_t[:], in_=idx_d[:])
            cw_t = []
            for k in range(KER):
                for eh in range(2):
                    for nh in range(2):
                        w = stat.tile([P, P], BF16, tag=f"cw{k}{eh}{nh}")
                        nc.sync.dma_start(out=w[:], in_=cw_d[(k * 2 + eh) * 2 + nh])
                        cw_t.append(w)
            cwf = lambda k, eh, nh: cw_t[(k * 2 + eh) * 2 + nh]
            cb_t = stat.tile([P, 2], F32, tag="cb")
            nc.sync.dma_start(out=cb_t[:], in_=cb_d[:])
            wih_t = []
            for g in range(4):
                for kh in range(2):
                    w = stat.tile([P, P], BF16, tag=f"wih{g}{kh}")
                    nc.sync.dma_start(out=w[:], in_=wih_d[g * 2 + kh])
                    wih_t.append(w)
            be_t = stat.tile([P, 4], F32, tag="be")
            nc.sync.dma_start(out=be_t[:], in_=be_d[:])
            u_t = []
            for g in range(4):
                w = stat.tile([P, P], BF16, tag=f"u{g}")
                nc.sync.dma_start(out=w[:], in_=u_d[g])
                u_t.append(w)
            eye_t = stat.tile([P, P], BF16, tag="eye")
            nc.sync.dma_start(out=eye_t[:], in_=eye_d[:])
            r0_t = stat.tile([P, BL], BF16, tag="r0")
            nc.sync.dma_start(out=r0_t[:], in_=r0_d[:])

            # Xp storage: col = t*128 + g*32 + b
            xp_sb = stat.tile([P, T * P], BF16, tag="xp")
            # view [p][t][g][b]
            xp_v = xp_sb[:].rearrange("p (t g b) -> p t g b", t=T, g=4, b=BL)

            # ================= PROLOGUE =================
            with tc.tile_pool(name="pgather", bufs=2) as pg, \
                 tc.tile_pool(name="pemb", bufs=2) as pe, \
                 tc.tile_pool(name="pconv", bufs=2) as pc, \
                 tc.tile_pool(name="ptr", bufs=2, space="PSUM") as ptr, \
                 tc.tile_pool(name="pcps", bufs=3, space="PSUM") as pcps, \
                 tc.tile_pool(name="pxps", bufs=2, space="PSUM") as pxps:
                for c in range(NCH):
                    G = pg.tile([P, CB * TQ * EMB], BF16, tag="G")
                    for j in range(CB * TQ):
                        nc.gpsimd.indirect_dma_start(
                            out=G[:, j * EMB:(j + 1) * EMB], out_offset=None,
                            in_=emb_d[:],
                            in_offset=bass.IndirectOffsetOnAxis(
                                ap=idx_t[:, c * CB * TQ + j:c * CB * TQ + j + 1],
                                axis=0),
                        )
                    embT = [pe.tile([P, CB * S], BF16, tag=f"embT{eh}", name=f"embT{eh}")
                            for eh in range(2)]
                    for j in range(CB * TQ):        # j = b_in*TQ + q
                        b_in, q = divmod(j, TQ)
                        for eh in range(2):
                            tp = ptr.tile([P, P], BF16, tag="tp")
                            nc.tensor.transpose(
                                out=tp[:], in_=G[:, j * EMB + eh * P: j * EMB + eh * P + P],
                                identity=eye_t[:])
                            dst = embT[eh][:, b_in * S + q * P: b_in * S + q * P + P]
                            if j % 2 == 0:
                                nc.vector.tensor_copy(out=dst, in_=tp[:])
                            else:
                                nc.scalar.copy(out=dst, in_=tp[:])
                    convT = [pc.tile([P, CB * T], BF16, tag=f"convT{nh}", name=f"convT{nh}")
                             for nh in range(2)]
                    for nh in range(2):
                        cv = convT[nh][:].rearrange("p (t b) -> p t b", t=T, b=CB)
                        for b_in in range(CB):
                            cps = pcps.tile([P, T], F32, tag="cps")
                            n_mm = 0
                            for k in range(KER):
                                for eh in range(2):
                                    nc.tensor.matmul(
                                        out=cps[:],
                                        lhsT=cwf(k, eh, nh)[:],
                                        rhs=embT[eh][:, b_in * S + k: b_in * S + k + T],
                                        start=(n_mm == 0), stop=(n_mm == 9))
                                    n_mm += 1
                            nc.scalar.activation(cv[:, :, b_in], cps[:], AF.Relu,
                                                 bias=cb_t[:, nh:nh + 1])
                    if debug and c == 0:
                        nc.sync.dma_start(out=g_dump[:], in_=G[:])
                        for eh in range(2):
                            nc.sync.dma_start(out=embT_dump[eh], in_=embT[eh][:])
                        pass
                    # Xp for this chunk: 4 n-chunks of 508 cols (127 t x 4 b)
                    for g in range(4):
                        for nck in range(4):
                            xps = pxps.tile([P, 127 * CB], F32, tag="xps")
                            for kh in range(2):
                                nc.tensor.matmul(
                                    out=xps[:],
                                    lhsT=wih_t[g * 2 + kh][:],
                                    rhs=convT[kh][:, nck * 127 * CB:(nck + 1) * 127 * CB],
                                    start=(kh == 0), stop=(kh == 1))
                            dst = xp_v[:, nck * 127:(nck + 1) * 127, g,
                                       c * CB:(c + 1) * CB]
                            nc.scalar.activation(dst, xps[:], AF.Identity, bias=be_t[:, g:g + 1])
                    if debug and c == 0:
                        for nh in range(2):
                            nc.sync.dma_start(out=convT_dump[nh], in_=convT[nh][:])

            if debug:
                pass  # dumps emitted inside prologue loop for c==0
            # ================= RECURRENCE =================
            with tc.tile_pool(name="rdyn", bufs=3) as dyn, \
                 tc.tile_pool(name="rps", bufs=3, space="PSUM") as rps:
                P_prev = stat.tile([P, BL], F32, tag="P_init")
                nc.vector.memset(P_prev[:], 0.0)
                r_prev = r0_t
                S_t = None
                sigP = None
                for t in range(T):
                    bank = rps.tile([P, P], F32, tag="bank")
                    nc.tensor.matmul(out=bank[:], lhsT=eye_t[:],
                                     rhs=xp_sb[:, t * P:(t + 1) * P],
                                     start=True, stop=False)
                    for g in range(4):
                        nc.tensor.matmul(out=bank[:, g * BL:(g + 1) * BL],
                                         lhsT=u_t[g][:], rhs=r_prev[:],
                                         start=False, stop=True)
                    S_t = dyn.tile([P, 96], F32, tag="S")
                    nc.scalar.activation(S_t[:], bank[:, 0:96], AF.Sigmoid)
                    So = dyn.tile([P, BL], F32, tag="So")
                    nc.scalar.activation(So[:], bank[:, 96:128], AF.Sigmoid)
                    if debug and t == 0:
                        nc.sync.dma_start(out=s0_dump[:, 0:96], in_=S_t[:])
                    t1 = dyn.tile([P, BL], F32, tag="t1")
                    nc.vector.scalar_tensor_tensor(
                        out=t1[:], in0=S_t[:, 64:96], scalar=0.5, in1=S_t[:, 32:64],
                        op0=OP.subtract, op1=OP.mult)
                    t2 = dyn.tile([P, BL], F32, tag="t2")
                    nc.vector.tensor_tensor(out=t2[:], in0=S_t[:, 0:32],
                                            in1=P_prev[:], op=OP.mult)
                    P_new = dyn.tile([P, BL], F32, tag="Pn")
                    nc.vector.tensor_tensor(out=P_new[:], in0=t1[:], in1=t2[:],
                                            op=OP.add)
                    sigP = dyn.tile([P, BL], F32, tag="sigP")
                    nc.scalar.activation(sigP[:], P_new[:], AF.Tanh, scale=2.0)
                    r_new = dyn.tile([P, BL], BF16, tag="r")
                    nc.vector.tensor_tensor(out=r_new[:], in0=sigP[:],
                                            in1=So[:], op=OP.mult)
                    if debug and t == 0:
                        rd = dyn.tile([P, BL], F32, tag="rdump")
                        nc.vector.tensor_copy(out=rd[:], in_=r_new[:])
                        nc.sync.dma_start(out=r1_dump[:], in_=rd[:])
                    r_prev, P_prev = r_new, P_new

                # exact final h = tanh(c) * sigma(o) in fp32
                hT = dyn.tile([P, BL], F32, tag="hT")
                nc.vector.tensor_tensor(out=hT[:], in0=sigP[:],
                                        in1=So[:], op=OP.mult)
                nc.sync.dma_start(out=hT_d[:], in_=hT[:])
                if debug:
                    nc.sync.dma_start(out=xp_dump[:], in_=xp_sb[:])

    nc.compile()
    return nc


def _prep_inputs(text, h_0, emb, conv_w, conv_b, w_ih, w_hh, b_ih, b_hh):
    bf = ml_dtypes.bfloat16
    text = np.asarray(text)
    h_0 = np.asarray(h_0, dtype=np.float32)
    emb = np.asarray(emb, dtype=np.float32)
    conv_w = np.asarray(conv_w, dtype=np.float32)
    conv_b = np.asarray(conv_b, dtype=np.float32)
    w_ih = np.asarray(w_ih, dtype=np.float32)
    w_hh = np.asarray(w_hh, dtype=np.float32)
    b_ih = np.asarray(b_ih, dtype=np.float32)
    b_hh = np.asarray(b_hh, dtype=np.float32)

    emb_bf = np.ascontiguousarray(emb.astype(bf))

    # conv weights: cw[k,eh,nh][e,n] = conv_w[nh*128+n, 0, k, eh*128+e]
    cw = conv_w[:, 0, :, :]                       # [NF, KER, EMB]
    cw = cw.transpose(1, 2, 0)                    # [KER, EMB, NF]
    cw = cw.reshape(KER, 2, P, 2, P)              # k, eh, e, nh, n
    cw = cw.transpose(0, 1, 3, 2, 4)              # k, eh, nh, e, n
    cw_in = np.ascontiguousarray(cw.reshape(KER * 4, P, P).astype(bf))
    cb_in = np.ascontiguousarray(conv_b.reshape(2, P).T)

    # gate reorder torch [i,f,g,o] -> ours [f,i,o,g]
    perm = [1, 0, 2, 3]
    wih_g = w_ih.reshape(4, P, NF)[perm]          # [4, 128, NF]
    whh_g = w_hh.reshape(4, P, HID)[perm]
    bias_g = (b_ih + b_hh).reshape(4, P)[perm]
    wih_g = wih_g * np.array([1, 1, 2, 1], np.float32)[:, None, None]
    bias_g = bias_g * np.array([1, 1, 2, 1], np.float32)[:, None]
    whh_g = whh_g * np.array([1, 1, 2, 1], np.float32)[:, None, None]

    # wih lhsT tiles: [g,kh][k,m] = wih_g[g, m, kh*128+k]
    wih_in = np.ascontiguousarray(
        wih_g.reshape(4, P, 2, P).transpose(0, 2, 3, 1)
        .reshape(8, P, P).astype(bf))
    be_in = np.ascontiguousarray(bias_g.reshape(4, P).T)
    # u lhsT tiles: [g][k,m] = whh_g[g, m, k]
    u_in = np.ascontiguousarray(whh_g.transpose(0, 2, 1).astype(bf))
    eye_in = np.eye(P, dtype=np.float32).astype(bf)

    text32 = text.astype(np.int32)
    in_maps = []
    for cidx in range(NC):
        tloc = text32[cidx * BL:(cidx + 1) * BL]           # [BL, S]
        # idx[p, b*TQ+q] = tloc[b, q*128+p]
        idx = np.ascontiguousarray(
            tloc.reshape(BL, TQ, P).transpose(2, 0, 1).reshape(P, BL * TQ))
        r0 = np.ascontiguousarray(
            h_0[0, cidx * BL:(cidx + 1) * BL].T.astype(bf))
        in_maps.append({
            "embt": emb_bf, "idx": idx, "cw": cw_in, "cb": cb_in,
            "wih": wih_in, "be": be_in, "u": u_in, "eye": eye_in, "r0": r0,
        })
    return in_maps


def kernel(**inputs) -> np.ndarray:
    global _PROGRAM
    if _PROGRAM is None:
        _PROGRAM = _build_program()
    in_maps = _prep_inputs(**inputs)
    res = run_bass_kernel_spmd(_PROGRAM, in_maps, core_ids=list(range(NC)))
    out = np.empty((B, HID), np.float32)
    for cidx in range(NC):
        out[cidx * BL:(cidx + 1) * BL] = res.results[cidx]["hT"].T
    return out

